# revision 26
# baseline (speedup 1.0000x reference)
"""Trainium2 Bass kernel for nn_Attention_33638183862624 (linear/Taylor-softmax
attention). Data-parallel over batch: 16 batches -> 8 NeuronCores, 2 each.

Math per batch (C=512, N=4096, CQK=64), x flattened to [C, N]:
  Q = Wq x + bq; K = Wk x + bk           (Q,K: [64, N])
  Qn = Q / ||Q||_col; Kn = K / ||K||_col
  ksum = sum_n Kn[:, n]                  [64]
  denom = N + Qn^T ksum; tailor = 1/denom
  V = Wv x + bv                          (never materialized; fused:)
  matrix = Kn V^T = (Kn x^T) Wv^T + ksum bv^T            [64, 512]
  vsum   = V 1_N  = Wv (x 1_N) + N bv                    [512]
  out[c,n] = gamma * tailor[n] * (vsum[c] + sum_m matrix[m,c] Qn[m,n])

Key device-side identities:
  * Q is kept RAW; with s = ksum^T Q_raw and nq = ||Q||_col:
      u := rq*tailor = 1/(N*nq + s), and tailor = nq*u.
    The final rhs is [Q_raw; nq] * broadcast(u) so one elementwise reciprocal
    on a [8, 512] tile per batch covers all of Q-normalize + tailor.
  * K is normalized in TRANSPOSED layout (per-partition 1/||K|| scalars,
    reciprocal on [128, 1] -> all 128 DVE lanes busy).
  * matrix/vsum/ksum come from one accumulated (Kn_ext @ x^T) product plus a
    small @ Wv^T stage; V itself is never computed.

All matmuls run as float32r (1-pass reduced-precision fp32, full PE rate at
free-dim 512). n-contractions use PE transposes of x and K chunks.
"""

import numpy as np

B, C, H, W = 16, 512, 64, 64
N = H * W          # 4096
CQK = C // 8       # 64
NCORES = 8
BLOC = B // NCORES  # 2 batches per core
NB = N // 512       # 8 n-chunks of 512
KC = C // 128       # 4 channel chunks of 128


# ---------------------------------------------------------------------------
# Walrus workaround: this container's walrus rejects >1 sync wait per
# instruction ("Too many sync wait commands"). (1) patch the TileContext tail
# drain to carry its waits on single-wait NOPs; (2) post-pass that rewrites
# any instruction with k>1 waits into k-1 single-wait NOPs + the instruction.
# ---------------------------------------------------------------------------

def _apply_tile_patches():
    import concourse.tile as tile
    from concourse import mybir
    from concourse.vector_clock import ScopedClock

    if getattr(tile.TileContext, "_drain_patched", False):
        return

    def _patched_drain_and_barrier(self, tick_clock, wait_clock):
        nop = self.nc.sync.nop(nofuse=True, hint="tail_drain_waits")
        wait_clock.add_sem_waits(
            nop.ins, ScopedClock({None: tick_clock.global_clock})
        )
        si = nop.ins.sync_info
        if si is not None and len(si.on_wait) > 1:
            waits = list(si.on_wait)
            nop.ins.sync_info = mybir.SyncInfo(on_wait=waits[:1], on_update=[])
            rest = waits[1:]
            while rest:
                n2 = self.nc.sync.nop(nofuse=True, hint="tail_drain_waits")
                n2.ins.sync_info = mybir.SyncInfo(on_wait=rest[:1], on_update=[])
                rest = rest[1:]
        self.nc.sync.drain()
        self.nc.all_engine_barrier()
        assert self.sems is not None
        popped = self.nc._tile_sem_poison_stack.pop()
        assert popped is self._sem_poison
        self.nc.clear_and_free_semaphores(list(self.sems.allocated().values()))
        self.nc.all_engine_barrier()

    tile.TileContext._drain_and_barrier = _patched_drain_and_barrier
    tile.TileContext._drain_patched = True


def _split_multi_waits(nc):
    from concourse import mybir

    counter = [0]
    for f in nc.m.functions:
        for bb in f.blocks:
            insts = bb.instructions
            if not any(
                i.sync_info is not None and len(i.sync_info.on_wait) > 1
                for i in insts
            ):
                continue
            new = []
            for ins in insts:
                si = ins.sync_info
                if si is not None and len(si.on_wait) > 1:
                    waits = list(si.on_wait)
                    for w in waits[:-1]:
                        counter[0] += 1
                        nop = mybir.InstNoOp(
                            name=f"I-wsplit-{counter[0]}", ins=[], outs=[]
                        )
                        nop.engine = ins.engine
                        nop.sync_info = mybir.SyncInfo(on_wait=[w], on_update=[])
                        new.append(nop)
                    ins.sync_info = mybir.SyncInfo(
                        on_wait=[waits[-1]], on_update=list(si.on_update)
                    )
                new.append(ins)
            bb.instructions = new


# ---------------------------------------------------------------------------
# Kernel body
# ---------------------------------------------------------------------------

def _build_module(use_bqk=True, use_bv=True):
    import concourse.bass as bass
    import concourse.tile as tile
    from concourse import mybir

    _apply_tile_patches()
    f32 = mybir.dt.float32
    f32r = mybir.dt.float32r
    alu = mybir.AluOpType
    r = lambda ap: ap.bitcast(f32r)

    nc = bass.Bass("TRN2", target_bir_lowering=False, debug=False)

    x_d = nc.dram_tensor("x", [BLOC, C, N], f32, kind="ExternalInput").ap()
    wqkt_d = nc.dram_tensor("wqkt", [128, KC, 128], f32, kind="ExternalInput").ap()
    wvt_d = nc.dram_tensor("wvt", [128, KC, 512], f32, kind="ExternalInput").ap()
    bqk_d = nc.dram_tensor("bqk", [1, 128], f32, kind="ExternalInput").ap()
    bv_d = nc.dram_tensor("bv", [1, 512], f32, kind="ExternalInput").ap()
    gam_d = nc.dram_tensor("gamma", [1, 1], f32, kind="ExternalInput").ap()
    sel8_d = nc.dram_tensor("sel8", [64, 64], f32, kind="ExternalInput").ap()
    selab_d = nc.dram_tensor("selab", [40, 520], f32, kind="ExternalInput").ap()
    onesn_d = nc.dram_tensor("onesn", [1, 512], f32, kind="ExternalInput").ap()
    onesc2_d = nc.dram_tensor("onesc2", [128, 2], f32, kind="ExternalInput").ap()
    ident_d = nc.dram_tensor("ident", [128, 128], f32, kind="ExternalInput").ap()
    out_d = nc.dram_tensor("out", [BLOC, C, N], f32, kind="ExternalOutput").ap()

    from contextlib import ExitStack

    with tile.TileContext(nc) as tc, ExitStack() as ctx, \
            nc.allow_low_precision(reason="float32r views are bit-compatible fp32"):
        consts = ctx.enter_context(tc.tile_pool(name="consts", bufs=1))
        xpool = ctx.enter_context(tc.tile_pool(name="xpool", bufs=2))
        batchp = ctx.enter_context(tc.tile_pool(name="batchp", bufs=2))
        work = ctx.enter_context(tc.tile_pool(name="work", bufs=3))
        outp = ctx.enter_context(tc.tile_pool(name="outp", bufs=4))
        pp_big = ctx.enter_context(tc.tile_pool(name="pp_big", bufs=5, space="PSUM"))
        pp_acc = ctx.enter_context(tc.tile_pool(name="pp_acc", bufs=2, space="PSUM"))
        pp_small = ctx.enter_context(
            tc.tile_pool(name="pp_small", bufs=1, space="PSUM")
        )

        # ---- constants ----
        wqkt = consts.tile([128, KC, 128], f32)
        nc.sync.dma_start(out=r(wqkt), in_=r(wqkt_d))
        wvt = consts.tile([128, KC, 512], f32)
        nc.sync.dma_start(out=r(wvt), in_=r(wvt_d))
        bqk = consts.tile([1, 128], f32)
        nc.sync.dma_start(out=r(bqk), in_=r(bqk_d))
        bv = consts.tile([1, 512], f32)
        nc.sync.dma_start(out=r(bv), in_=r(bv_d))
        sel8 = consts.tile([64, 64], f32)
        nc.sync.dma_start(out=r(sel8), in_=r(sel8_d))
        selab = consts.tile([40, 520], f32)
        nc.sync.dma_start(out=r(selab), in_=r(selab_d))
        onesn = consts.tile([1, 512], f32)
        nc.sync.dma_start(out=r(onesn), in_=r(onesn_d))
        onesc2 = consts.tile([128, 2], f32)
        nc.sync.dma_start(out=r(onesc2), in_=r(onesc2_d))
        ident = consts.tile([128, 128], f32)
        nc.sync.dma_start(out=r(ident), in_=r(ident_d))
        warm_ps = pp_big.tile([128, 512], f32, tag="big", name="warm_ps")
        for _ in range(24):
            nc.tensor.matmul(
                warm_ps, r(ident), r(wvt[:, 0, :]), start=True, stop=True
            )
        gam128 = consts.tile([128, 1], f32)
        nc.sync.dma_start(
            out=gam128,
            in_=bass.AP(
                tensor=gam_d.tensor, offset=gam_d.offset,
                ap=[[0, 128], [1, 1]],
            ),
        )

        def alloc_state(b):
            st = {}
            st["q_raw"] = batchp.tile([65, N], f32, tag="q_raw",
                                      name=f"q_raw{b}")
            nc.gpsimd.memset(st["q_raw"][64:65, :], 1.0)
            st["ks8"] = batchp.tile([64, 64], f32, tag="ks8", name=f"ks8_{b}")
            nc.gpsimd.memset(st["ks8"], 0.0)
            st["ks_parts"] = batchp.tile([65, NB], f32, tag="ks_parts",
                                         name=f"ks_parts{b}")
            st["ksum_full"] = batchp.tile([65, 1], f32, tag="ksum_full",
                                          name=f"ksum_full{b}")
            st["ksumn_row"] = batchp.tile([1, 65], f32, tag="ksumn_row",
                                          name=f"ksumn_row{b}")
            st["p_sb"] = batchp.tile([65, 512], f32, tag="p_sb",
                                     name=f"p_sb{b}")
            st["pt_sb"] = batchp.tile([128, KC, 65], f32, tag="pt_sb",
                                      name=f"pt_sb{b}")
            st["mat_sb"] = batchp.tile([65, 512], f32, tag="mat_sb",
                                       name=f"mat_sb{b}")
            st["ut"] = batchp.tile([40, 512], f32, tag="ut", name=f"ut{b}")
            st["p_ps"] = pp_acc.tile([65, 512], f32, tag="acc",
                                     name=f"p_ps{b}")
            st["n2q8_ps"] = pp_acc.tile([8, 512], f32, tag="acc",
                                        name=f"n2q8_ps{b}")
            st["xh"] = {}
            st["xt"] = {}
            st["knt"] = {}
            st["qns"] = {}
            return st

        def emit_A_chunk(b, st, nb):
            half, col = nb // 4, (nb % 4) * 512
            if nb % 4 == 0:
                for k in range(KC):
                    st["xh"][k] = xpool.tile([128, 2048], f32, tag=f"x{k}",
                                             name=f"xh{k}_{b}_{half}")
                    for piece in range(2):
                        nc.sync.dma_start(
                            out=r(st["xh"][k][:, 1024 * piece:1024 * (piece + 1)]),
                            in_=r(x_d[b, 128 * k:128 * (k + 1),
                                      2048 * half + 1024 * piece:
                                      2048 * half + 1024 * (piece + 1)]),
                        )
            xh = st["xh"]

            # QK = Wqk x + bqk -> psum [128, 512] (rows 0-63 Q, 64-127 K)
            qk_ps = pp_big.tile([128, 512], f32, tag="big", name=f"qk{b}_{nb}")
            for k in range(KC):
                nc.tensor.matmul(
                    qk_ps, r(wqkt[:, k, :]), r(xh[k][:, col:col + 512]),
                    start=(k == 0), stop=(k == KC - 1 and not use_bqk),
                )
            if use_bqk:
                nc.tensor.matmul(qk_ps, r(bqk), r(onesn), start=False, stop=True)

            # stash raw Q; K to sbuf for transposing
            nc.vector.tensor_copy(
                out=r(st["q_raw"][0:64, 512 * nb:512 * (nb + 1)]),
                in_=qk_ps[0:64, :],
            )
            k_sb = work.tile([64, 512], f32, tag="k_sb", name=f"k_sb{b}_{nb}")
            nc.vector.tensor_copy(out=r(k_sb), in_=qk_ps[64:128, :])
            sq_sb = work.tile([64, 512], f32, tag="sq", name=f"sq{b}_{nb}")
            nc.scalar.square(out=r(sq_sb), in_=qk_ps[0:64, :])

            # x^T chunks (depend only on xh -> keep PE stream dense here
            # while DVE/ACT produce k_sb/sq)
            st["xt"][nb] = []
            for j in range(4):
                xt_ps = pp_big.tile([128, 512], f32, tag="big",
                                    name=f"xt{b}_{nb}_{j}")
                for k in range(KC):
                    nc.tensor.transpose(
                        r(xt_ps[:, 128 * k:128 * (k + 1)]),
                        r(xh[k][:, col + 128 * j:col + 128 * (j + 1)]),
                        r(ident),
                    )
                xt_sb = work.tile([128, 512], f32, tag="xt", bufs=9,
                                  name=f"xtsb{b}_{nb}_{j}")
                if j % 2 == 0:
                    nc.vector.tensor_copy(out=r(xt_sb), in_=xt_ps)
                else:
                    nc.scalar.copy(out=r(xt_sb), in_=xt_ps)
                st["xt"][nb].append(xt_sb)

            # K^T chunks (raw), then normalize per-partition
            kt_ps = pp_big.tile([128, 256], f32, tag="big", name=f"kt{b}_{nb}")
            for j in range(4):
                nc.tensor.transpose(
                    r(kt_ps[:, 64 * j:64 * (j + 1)]),
                    r(k_sb[:, 128 * j:128 * (j + 1)]),
                    r(ident[0:64, 0:64]),
                )
            # Q column norms^2 -> accumulate into row nb of n2q8_ps
            nc.tensor.matmul(
                st["n2q8_ps"], r(sel8[:, 8 * nb:8 * (nb + 1)]), r(sq_sb),
                start=(nb == 0), stop=(nb == NB - 1), skip_group_check=True,
            )
            knt_raw = work.tile([128, 256], f32, tag="knt_raw",
                                name=f"knt_raw{b}_{nb}")
            nc.scalar.copy(out=knt_raw, in_=kt_ps)
            knt_sb = work.tile([128, 4, 65], f32, tag="knt", bufs=4,
                               name=f"knt{b}_{nb}")
            for j in range(4):
                ksq = work.tile([128, 64], f32, tag="ksq", name=f"ksq{b}_{nb}_{j}")
                nc.scalar.square(out=ksq, in_=knt_raw[:, 64 * j:64 * (j + 1)])
                nk2 = work.tile([128, 1], f32, tag="nk2", name=f"nk2{b}_{nb}_{j}")
                nc.vector.reduce_sum(out=nk2, in_=ksq, axis=mybir.AxisListType.X)
                nkt = work.tile([128, 1], f32, tag="nkt", name=f"nkt{b}_{nb}_{j}")
                nc.scalar.sqrt(out=nkt, in_=nk2)
                rkt = work.tile([128, 1], f32, tag="rkt", name=f"rkt{b}_{nb}_{j}")
                nc.vector.reciprocal(out=rkt, in_=nkt)
                nc.vector.tensor_scalar_mul(
                    out=r(knt_sb[:, j, 0:64]),
                    in0=knt_raw[:, 64 * j:64 * (j + 1)], scalar1=rkt,
                )
            nc.gpsimd.memset(knt_sb[:, :, 64:65], 1.0)
            st["knt"][nb] = knt_sb

        def emit_P_chunk(b, st, nb):
            # deferred one chunk so the knt/xt producer chains have slack
            knt_sb = st["knt"].pop(nb)
            xts = st["xt"].pop(nb)
            ks_ps = pp_small.tile([65, 2], f32, tag="small", name=f"ksp{b}_{nb}")
            for j in range(4):
                nc.tensor.matmul(
                    st["p_ps"], r(knt_sb[:, j, :]), r(xts[j]),
                    start=(nb == 0 and j == 0),
                    stop=(nb == NB - 1 and j == 3),
                    skip_group_check=True,
                )
                nc.tensor.matmul(
                    ks_ps, r(knt_sb[:, j, :]), r(onesc2),
                    start=(j == 0), stop=(j == 3),
                    skip_group_check=True,
                )
            nc.vector.tensor_copy(
                out=st["ks_parts"][:, nb:nb + 1], in_=ks_ps[:, 0:1]
            )

        def emit_A2(b, st):
            q_raw, ks8 = st["q_raw"], st["ks8"]
            ksum_full, mat_sb = st["ksum_full"], st["mat_sb"]
            nc.vector.reduce_sum(
                out=r(ksum_full), in_=st["ks_parts"], axis=mybir.AxisListType.X
            )
            if use_bv:
                ksr_ps = pp_small.tile([1, 66], f32, tag="small",
                                       name=f"ksr{b}")
                nc.tensor.matmul(
                    ksr_ps, r(ksum_full), r(ident[0:65, 0:66]),
                    start=True, stop=True,
                )
                nc.vector.tensor_copy(
                    out=r(st["ksumn_row"]), in_=ksr_ps[0:1, 0:65]
                )

            nc.vector.tensor_copy(out=r(st["p_sb"]), in_=st["p_ps"])
            pt_ps = pp_small.tile([128, 264], f32, tag="small", name=f"pt{b}")
            for k in range(KC):
                nc.tensor.transpose(
                    r(pt_ps[:, 66 * k:66 * (k + 1)]),
                    r(st["p_sb"][:, 128 * k:128 * (k + 1)]),
                    r(ident[0:65, 0:66]),
                )
            nc.vector.tensor_copy(
                out=r(st["pt_sb"]),
                in_=pt_ps[:].rearrange("p (k c) -> p k c", c=66)[:, :, 0:65],
            )
            mat_ps = pp_acc.tile([65, 512], f32, tag="acc", name=f"mat_ps{b}")
            for k in range(KC):
                nc.tensor.matmul(
                    mat_ps, r(st["pt_sb"][:, k, :]), r(wvt[:, k, :]),
                    start=(k == 0), stop=(k == KC - 1 and not use_bv),
                    skip_group_check=True,
                )
            if use_bv:
                nc.tensor.matmul(
                    mat_ps, r(st["ksumn_row"]), r(bv), start=False, stop=True,
                    skip_group_check=True,
                )
            # gamma folded into matrix_ext during the psum->sbuf move
            nc.vector.tensor_scalar_mul(
                out=r(mat_sb), in0=mat_ps, scalar1=gam128[0:65, :]
            )

            # ks8: column nb holds ksum in slot nb of each 8-block
            for nb in range(NB):
                nc.vector.tensor_copy(
                    out=r(ks8[:, 8 * nb + nb:8 * nb + nb + 1]),
                    in_=ksum_full[0:64, :],
                )
            # s8[i, :] = ksum^T Q_raw(chunk i), stacked via one-hot lhsT
            s8_ps = pp_acc.tile([8, 512], f32, tag="acc", name=f"s8{b}")
            for nb in range(NB):
                sl = slice(512 * nb, 512 * (nb + 1))
                nc.tensor.matmul(
                    s8_ps, r(ks8[:, 8 * nb:8 * (nb + 1)]), r(q_raw[0:64, sl]),
                    start=(nb == 0), stop=(nb == NB - 1), skip_group_check=True,
                )

            # u = 1/(N*nq + s); tailor = nq*u
            # ut: rows 0-7 = u per chunk, rows 32-39 = tailor per chunk
            nq8 = work.tile([8, 512], f32, tag="nq8", name=f"nq8_{b}")
            nc.scalar.sqrt(out=nq8, in_=st["n2q8_ps"])
            t1 = work.tile([8, 512], f32, tag="t1", name=f"t1_{b}")
            nc.vector.scalar_tensor_tensor(
                out=t1, in0=nq8, scalar=float(N), in1=s8_ps,
                op0=alu.mult, op1=alu.add,
            )
            ut = st["ut"]
            nc.vector.reciprocal(out=r(ut[0:8, :]), in_=t1)
            nc.vector.tensor_mul(out=r(ut[32:40, :]), in0=nq8, in1=ut[0:8, :])

        def emit_B_t2(b, st, nb):
            sl = slice(512 * nb, 512 * (nb + 1))
            # T2 rows 0-63 = u(chunk nb), row 64 = tailor(chunk nb)
            t2_ps = pp_big.tile([65, 512], f32, tag="big", name=f"t2_{b}_{nb}")
            nc.tensor.matmul(
                t2_ps, r(selab[:, 65 * nb:65 * (nb + 1)]), r(st["ut"]),
                start=True, stop=True,
            )
            qns = work.tile([65, 512], f32, tag="qns", bufs=4,
                            name=f"qns{b}_{nb}")
            nc.vector.tensor_mul(out=r(qns), in0=st["q_raw"][:, sl], in1=t2_ps)
            st["qns"][nb] = qns

        def emit_B_out(b, st, nb):
            sl = slice(512 * nb, 512 * (nb + 1))
            qns = st["qns"].pop(nb)
            for cb in range(KC):
                o_ps = pp_big.tile([128, 512], f32, tag="big",
                                   name=f"o_ps{b}_{nb}_{cb}")
                nc.tensor.matmul(
                    o_ps, r(st["mat_sb"][:, 128 * cb:128 * (cb + 1)]), r(qns),
                    start=True, stop=True,
                )
                o_sb = outp.tile([128, 512], f32, tag="o",
                                 name=f"o_sb{b}_{nb}_{cb}")
                nc.scalar.copy(out=o_sb, in_=o_ps)
                nc.sync.dma_start(
                    out=out_d[b, 128 * cb:128 * (cb + 1), sl], in_=o_sb
                )

        # Software pipeline: batch b stage A interleaves with batch b-1
        # stage B; P/ks and out matmuls trail their producers by one chunk so
        # the in-order PE stream never waits on DVE/ACT chains.
        states = {}
        for b in range(BLOC):
            states[b] = alloc_state(b)
            for nb in range(NB):
                emit_A_chunk(b, states[b], nb)
                if nb > 0:
                    emit_P_chunk(b, states[b], nb - 1)
                if b > 0:
                    emit_B_t2(b - 1, states[b - 1], nb)
                    if nb > 0:
                        emit_B_out(b - 1, states[b - 1], nb - 1)
            emit_P_chunk(b, states[b], NB - 1)
            if b > 0:
                emit_B_out(b - 1, states[b - 1], NB - 1)
            emit_A2(b, states[b])
        stl = states[BLOC - 1]
        for nb in range(NB):
            emit_B_t2(BLOC - 1, stl, nb)
            if nb > 0:
                emit_B_out(BLOC - 1, stl, nb - 1)
        emit_B_out(BLOC - 1, stl, NB - 1)

    _split_multi_waits(nc)
    return nc


_CACHE = {}


def _get_module(use_bqk, use_bv):
    key = (use_bqk, use_bv)
    if key not in _CACHE:
        _CACHE[key] = _build_module(*key)
    return _CACHE[key]


def _host_inputs(x, Wq, bq, Wk, bk, Wv, bv, gamma):
    x = np.ascontiguousarray(np.asarray(x, dtype=np.float32)).reshape(B, C, N)
    Wq = np.asarray(Wq, dtype=np.float32)
    Wk = np.asarray(Wk, dtype=np.float32)
    Wv = np.asarray(Wv, dtype=np.float32)
    bq = np.asarray(bq, dtype=np.float32)
    bk = np.asarray(bk, dtype=np.float32)
    bvv = np.asarray(bv, dtype=np.float32)
    gamma = np.asarray(gamma, dtype=np.float32)

    wqk = np.concatenate([Wq, Wk], axis=0)            # [128, 512]
    wqkt = np.ascontiguousarray(
        wqk.T.reshape(KC, 128, 128).transpose(1, 0, 2)
    )                                                 # [128, KC, 128]
    wvt = np.ascontiguousarray(
        Wv.T.reshape(KC, 128, 512).transpose(1, 0, 2)
    )                                                 # [128, KC, 512]
    bqkr = np.concatenate([bq, bk]).reshape(1, 128)
    bvr = bvv.reshape(1, 512)
    gam = gamma.reshape(1, 1).astype(np.float32)
    sel8 = np.zeros((64, 64), np.float32)
    for nb in range(8):
        sel8[:, 8 * nb + nb] = 1.0
    selab = np.zeros((40, 520), np.float32)
    for nb in range(8):
        selab[nb, 65 * nb:65 * nb + 64] = 1.0
        selab[32 + nb, 65 * nb + 64] = 1.0
    onesn = np.ones((1, 512), np.float32)
    onesc2 = np.ones((128, 2), np.float32)
    ident = np.eye(128, dtype=np.float32)

    shared = dict(
        wqkt=wqkt, wvt=wvt, bqk=bqkr, bv=bvr, gamma=gam,
        sel8=sel8, selab=selab, onesn=onesn, onesc2=onesc2,
        ident=ident,
    )
    in_maps = []
    for c in range(NCORES):
        m = dict(shared)
        m["x"] = np.ascontiguousarray(x[c * BLOC:(c + 1) * BLOC])
        in_maps.append(m)
    return in_maps


def run_on_device(in_maps, **kw):
    from concourse.bass_utils import run_bass_kernel_spmd

    m = in_maps[0]
    use_bqk = bool(np.any(m["bqk"]))
    use_bv = bool(np.any(m["bv"]))
    nc = _get_module(use_bqk, use_bv)
    return run_bass_kernel_spmd(nc, in_maps, core_ids=list(range(NCORES)), **kw)


def kernel(x, Wq, bq, Wk, bk, Wv, bv, gamma):
    in_maps = _host_inputs(x, Wq, bq, Wk, bk, Wv, bv, gamma)
    res = run_on_device(in_maps)
    out = np.concatenate([r["out"] for r in res.results], axis=0)
    return out.reshape(B, C, H, W).astype(np.float32)


# revision 27
# speedup vs baseline: 1.0055x; 1.0055x over previous
"""Trainium2 Bass kernel for nn_Attention_33638183862624 (linear/Taylor-softmax
attention). Data-parallel over batch: 16 batches -> 8 NeuronCores, 2 each.

Math per batch (C=512, N=4096, CQK=64), x flattened to [C, N]:
  Q = Wq x + bq; K = Wk x + bk           (Q,K: [64, N])
  Qn = Q / ||Q||_col; Kn = K / ||K||_col
  ksum = sum_n Kn[:, n]                  [64]
  denom = N + Qn^T ksum; tailor = 1/denom
  V = Wv x + bv                          (never materialized; fused:)
  matrix = Kn V^T = (Kn x^T) Wv^T + ksum bv^T            [64, 512]
  vsum   = V 1_N  = Wv (x 1_N) + N bv                    [512]
  out[c,n] = gamma * tailor[n] * (vsum[c] + sum_m matrix[m,c] Qn[m,n])

Key device-side identities:
  * Q is kept RAW; with s = ksum^T Q_raw and nq = ||Q||_col:
      u := rq*tailor = 1/(N*nq + s), and tailor = nq*u.
    The final rhs is [Q_raw; nq] * broadcast(u) so one elementwise reciprocal
    on a [8, 512] tile per batch covers all of Q-normalize + tailor.
  * K is normalized in TRANSPOSED layout (per-partition 1/||K|| scalars,
    reciprocal on [128, 1] -> all 128 DVE lanes busy).
  * matrix/vsum/ksum come from one accumulated (Kn_ext @ x^T) product plus a
    small @ Wv^T stage; V itself is never computed.

All matmuls run as float32r (1-pass reduced-precision fp32, full PE rate at
free-dim 512). n-contractions use PE transposes of x and K chunks.
"""

import numpy as np

B, C, H, W = 16, 512, 64, 64
N = H * W          # 4096
CQK = C // 8       # 64
NCORES = 8
BLOC = B // NCORES  # 2 batches per core
NB = N // 512       # 8 n-chunks of 512
KC = C // 128       # 4 channel chunks of 128


# ---------------------------------------------------------------------------
# Walrus workaround: this container's walrus rejects >1 sync wait per
# instruction ("Too many sync wait commands"). (1) patch the TileContext tail
# drain to carry its waits on single-wait NOPs; (2) post-pass that rewrites
# any instruction with k>1 waits into k-1 single-wait NOPs + the instruction.
# ---------------------------------------------------------------------------

def _apply_tile_patches():
    import concourse.tile as tile
    from concourse import mybir
    from concourse.vector_clock import ScopedClock

    if getattr(tile.TileContext, "_drain_patched", False):
        return

    def _patched_drain_and_barrier(self, tick_clock, wait_clock):
        nop = self.nc.sync.nop(nofuse=True, hint="tail_drain_waits")
        wait_clock.add_sem_waits(
            nop.ins, ScopedClock({None: tick_clock.global_clock})
        )
        si = nop.ins.sync_info
        if si is not None and len(si.on_wait) > 1:
            waits = list(si.on_wait)
            nop.ins.sync_info = mybir.SyncInfo(on_wait=waits[:1], on_update=[])
            rest = waits[1:]
            while rest:
                n2 = self.nc.sync.nop(nofuse=True, hint="tail_drain_waits")
                n2.ins.sync_info = mybir.SyncInfo(on_wait=rest[:1], on_update=[])
                rest = rest[1:]
        self.nc.sync.drain()
        self.nc.all_engine_barrier()
        assert self.sems is not None
        popped = self.nc._tile_sem_poison_stack.pop()
        assert popped is self._sem_poison
        self.nc.clear_and_free_semaphores(list(self.sems.allocated().values()))
        self.nc.all_engine_barrier()

    tile.TileContext._drain_and_barrier = _patched_drain_and_barrier
    tile.TileContext._drain_patched = True


def _split_multi_waits(nc):
    from concourse import mybir

    counter = [0]
    for f in nc.m.functions:
        for bb in f.blocks:
            insts = bb.instructions
            if not any(
                i.sync_info is not None and len(i.sync_info.on_wait) > 1
                for i in insts
            ):
                continue
            new = []
            for ins in insts:
                si = ins.sync_info
                if si is not None and len(si.on_wait) > 1:
                    waits = list(si.on_wait)
                    for w in waits[:-1]:
                        counter[0] += 1
                        nop = mybir.InstNoOp(
                            name=f"I-wsplit-{counter[0]}", ins=[], outs=[]
                        )
                        nop.engine = ins.engine
                        nop.sync_info = mybir.SyncInfo(on_wait=[w], on_update=[])
                        new.append(nop)
                    ins.sync_info = mybir.SyncInfo(
                        on_wait=[waits[-1]], on_update=list(si.on_update)
                    )
                new.append(ins)
            bb.instructions = new


# ---------------------------------------------------------------------------
# Kernel body
# ---------------------------------------------------------------------------

def _build_module(use_bqk=True, use_bv=True):
    import concourse.bass as bass
    import concourse.tile as tile
    from concourse import mybir

    _apply_tile_patches()
    f32 = mybir.dt.float32
    f32r = mybir.dt.float32r
    alu = mybir.AluOpType
    r = lambda ap: ap.bitcast(f32r)

    nc = bass.Bass("TRN2", target_bir_lowering=False, debug=False)

    x_d = nc.dram_tensor("x", [BLOC, C, N], f32, kind="ExternalInput").ap()
    wqkt_d = nc.dram_tensor("wqkt", [128, KC, 128], f32, kind="ExternalInput").ap()
    wvt_d = nc.dram_tensor("wvt", [128, KC, 512], f32, kind="ExternalInput").ap()
    bqk_d = nc.dram_tensor("bqk", [1, 128], f32, kind="ExternalInput").ap()
    bv_d = nc.dram_tensor("bv", [1, 512], f32, kind="ExternalInput").ap()
    gam_d = nc.dram_tensor("gamma", [1, 1], f32, kind="ExternalInput").ap()
    sel8_d = nc.dram_tensor("sel8", [64, 64], f32, kind="ExternalInput").ap()
    selab_d = nc.dram_tensor("selab", [40, 520], f32, kind="ExternalInput").ap()
    onesn_d = nc.dram_tensor("onesn", [1, 512], f32, kind="ExternalInput").ap()
    onesc2_d = nc.dram_tensor("onesc2", [128, 2], f32, kind="ExternalInput").ap()
    ident_d = nc.dram_tensor("ident", [128, 128], f32, kind="ExternalInput").ap()
    out_d = nc.dram_tensor("out", [BLOC, C, N], f32, kind="ExternalOutput").ap()

    from contextlib import ExitStack

    with tile.TileContext(nc) as tc, ExitStack() as ctx, \
            nc.allow_low_precision(reason="float32r views are bit-compatible fp32"):
        consts = ctx.enter_context(tc.tile_pool(name="consts", bufs=1))
        xpool = ctx.enter_context(tc.tile_pool(name="xpool", bufs=2))
        batchp = ctx.enter_context(tc.tile_pool(name="batchp", bufs=2))
        work = ctx.enter_context(tc.tile_pool(name="work", bufs=3))
        outp = ctx.enter_context(tc.tile_pool(name="outp", bufs=4))
        pp_big = ctx.enter_context(tc.tile_pool(name="pp_big", bufs=5, space="PSUM"))
        pp_acc = ctx.enter_context(tc.tile_pool(name="pp_acc", bufs=2, space="PSUM"))
        pp_small = ctx.enter_context(
            tc.tile_pool(name="pp_small", bufs=1, space="PSUM")
        )

        # ---- constants ----
        wqkt = consts.tile([128, KC, 128], f32)
        nc.sync.dma_start(out=r(wqkt), in_=r(wqkt_d))
        wvt = consts.tile([128, KC, 512], f32)
        nc.sync.dma_start(out=r(wvt), in_=r(wvt_d))
        bqk = consts.tile([1, 128], f32)
        nc.sync.dma_start(out=r(bqk), in_=r(bqk_d))
        bv = consts.tile([1, 512], f32)
        nc.sync.dma_start(out=r(bv), in_=r(bv_d))
        sel8 = consts.tile([64, 64], f32)
        nc.sync.dma_start(out=r(sel8), in_=r(sel8_d))
        selab = consts.tile([40, 520], f32)
        nc.sync.dma_start(out=r(selab), in_=r(selab_d))
        onesn = consts.tile([1, 512], f32)
        nc.sync.dma_start(out=r(onesn), in_=r(onesn_d))
        onesc2 = consts.tile([128, 2], f32)
        nc.sync.dma_start(out=r(onesc2), in_=r(onesc2_d))
        ident = consts.tile([128, 128], f32)
        nc.sync.dma_start(out=r(ident), in_=r(ident_d))
        gam128 = consts.tile([128, 1], f32)
        nc.sync.dma_start(
            out=gam128,
            in_=bass.AP(
                tensor=gam_d.tensor, offset=gam_d.offset,
                ap=[[0, 128], [1, 1]],
            ),
        )

        def alloc_state(b):
            st = {}
            st["q_raw"] = batchp.tile([65, N], f32, tag="q_raw",
                                      name=f"q_raw{b}")
            nc.gpsimd.memset(st["q_raw"][64:65, :], 1.0)
            st["ks8"] = batchp.tile([64, 64], f32, tag="ks8", name=f"ks8_{b}")
            nc.gpsimd.memset(st["ks8"], 0.0)
            st["ks_parts"] = batchp.tile([65, NB], f32, tag="ks_parts",
                                         name=f"ks_parts{b}")
            st["ksum_full"] = batchp.tile([65, 1], f32, tag="ksum_full",
                                          name=f"ksum_full{b}")
            st["ksumn_row"] = batchp.tile([1, 65], f32, tag="ksumn_row",
                                          name=f"ksumn_row{b}")
            st["p_sb"] = batchp.tile([65, 512], f32, tag="p_sb",
                                     name=f"p_sb{b}")
            st["pt_sb"] = batchp.tile([128, KC, 65], f32, tag="pt_sb",
                                      name=f"pt_sb{b}")
            st["mat_sb"] = batchp.tile([65, 512], f32, tag="mat_sb",
                                       name=f"mat_sb{b}")
            st["ut"] = batchp.tile([40, 512], f32, tag="ut", name=f"ut{b}")
            st["p_ps"] = pp_acc.tile([65, 512], f32, tag="acc",
                                     name=f"p_ps{b}")
            st["n2q8_ps"] = pp_acc.tile([8, 512], f32, tag="acc",
                                        name=f"n2q8_ps{b}")
            st["xh"] = {}
            st["xt"] = {}
            st["knt"] = {}
            st["qns"] = {}
            return st

        def emit_A_chunk(b, st, nb):
            half, col = nb // 4, (nb % 4) * 512
            if nb % 4 == 0:
                for k in range(KC):
                    st["xh"][k] = xpool.tile([128, 2048], f32, tag=f"x{k}",
                                             name=f"xh{k}_{b}_{half}")
                    for piece in range(2):
                        nc.sync.dma_start(
                            out=r(st["xh"][k][:, 1024 * piece:1024 * (piece + 1)]),
                            in_=r(x_d[b, 128 * k:128 * (k + 1),
                                      2048 * half + 1024 * piece:
                                      2048 * half + 1024 * (piece + 1)]),
                        )
            xh = st["xh"]

            # QK = Wqk x + bqk -> psum [128, 512] (rows 0-63 Q, 64-127 K)
            qk_ps = pp_big.tile([128, 512], f32, tag="big", name=f"qk{b}_{nb}")
            for k in range(KC):
                nc.tensor.matmul(
                    qk_ps, r(wqkt[:, k, :]), r(xh[k][:, col:col + 512]),
                    start=(k == 0), stop=(k == KC - 1 and not use_bqk),
                )
            if use_bqk:
                nc.tensor.matmul(qk_ps, r(bqk), r(onesn), start=False, stop=True)

            # stash raw Q; K to sbuf for transposing
            nc.vector.tensor_copy(
                out=r(st["q_raw"][0:64, 512 * nb:512 * (nb + 1)]),
                in_=qk_ps[0:64, :],
            )
            k_sb = work.tile([64, 512], f32, tag="k_sb", name=f"k_sb{b}_{nb}")
            nc.vector.tensor_copy(out=r(k_sb), in_=qk_ps[64:128, :])
            sq_sb = work.tile([64, 512], f32, tag="sq", name=f"sq{b}_{nb}")
            nc.scalar.square(out=r(sq_sb), in_=qk_ps[0:64, :])

            # x^T chunks (depend only on xh -> keep PE stream dense here
            # while DVE/ACT produce k_sb/sq)
            st["xt"][nb] = []
            for j in range(4):
                xt_ps = pp_big.tile([128, 512], f32, tag="big",
                                    name=f"xt{b}_{nb}_{j}")
                for k in range(KC):
                    nc.tensor.transpose(
                        r(xt_ps[:, 128 * k:128 * (k + 1)]),
                        r(xh[k][:, col + 128 * j:col + 128 * (j + 1)]),
                        r(ident),
                    )
                xt_sb = work.tile([128, 512], f32, tag="xt", bufs=9,
                                  name=f"xtsb{b}_{nb}_{j}")
                if j % 2 == 0:
                    nc.vector.tensor_copy(out=r(xt_sb), in_=xt_ps)
                else:
                    nc.scalar.copy(out=r(xt_sb), in_=xt_ps)
                st["xt"][nb].append(xt_sb)

            # K^T chunks (raw), then normalize per-partition
            kt_ps = pp_big.tile([128, 256], f32, tag="big", name=f"kt{b}_{nb}")
            for j in range(4):
                nc.tensor.transpose(
                    r(kt_ps[:, 64 * j:64 * (j + 1)]),
                    r(k_sb[:, 128 * j:128 * (j + 1)]),
                    r(ident[0:64, 0:64]),
                )
            # Q column norms^2 -> accumulate into row nb of n2q8_ps
            nc.tensor.matmul(
                st["n2q8_ps"], r(sel8[:, 8 * nb:8 * (nb + 1)]), r(sq_sb),
                start=(nb == 0), stop=(nb == NB - 1), skip_group_check=True,
            )
            knt_raw = work.tile([128, 256], f32, tag="knt_raw",
                                name=f"knt_raw{b}_{nb}")
            nc.scalar.copy(out=knt_raw, in_=kt_ps)
            knt_sb = work.tile([128, 4, 65], f32, tag="knt", bufs=4,
                               name=f"knt{b}_{nb}")
            for j in range(4):
                ksq = work.tile([128, 64], f32, tag="ksq", name=f"ksq{b}_{nb}_{j}")
                nc.scalar.square(out=ksq, in_=knt_raw[:, 64 * j:64 * (j + 1)])
                nk2 = work.tile([128, 1], f32, tag="nk2", name=f"nk2{b}_{nb}_{j}")
                nc.vector.reduce_sum(out=nk2, in_=ksq, axis=mybir.AxisListType.X)
                nkt = work.tile([128, 1], f32, tag="nkt", name=f"nkt{b}_{nb}_{j}")
                nc.scalar.sqrt(out=nkt, in_=nk2)
                rkt = work.tile([128, 1], f32, tag="rkt", name=f"rkt{b}_{nb}_{j}")
                nc.vector.reciprocal(out=rkt, in_=nkt)
                nc.vector.tensor_scalar_mul(
                    out=r(knt_sb[:, j, 0:64]),
                    in0=knt_raw[:, 64 * j:64 * (j + 1)], scalar1=rkt,
                )
            nc.gpsimd.memset(knt_sb[:, :, 64:65], 1.0)
            st["knt"][nb] = knt_sb

        def emit_P_chunk(b, st, nb):
            # deferred one chunk so the knt/xt producer chains have slack
            knt_sb = st["knt"].pop(nb)
            xts = st["xt"].pop(nb)
            ks_ps = pp_small.tile([65, 2], f32, tag="small", name=f"ksp{b}_{nb}")
            for j in range(4):
                nc.tensor.matmul(
                    st["p_ps"], r(knt_sb[:, j, :]), r(xts[j]),
                    start=(nb == 0 and j == 0),
                    stop=(nb == NB - 1 and j == 3),
                    skip_group_check=True,
                )
                nc.tensor.matmul(
                    ks_ps, r(knt_sb[:, j, :]), r(onesc2),
                    start=(j == 0), stop=(j == 3),
                    skip_group_check=True,
                )
            nc.vector.tensor_copy(
                out=st["ks_parts"][:, nb:nb + 1], in_=ks_ps[:, 0:1]
            )

        def emit_A2(b, st):
            q_raw, ks8 = st["q_raw"], st["ks8"]
            ksum_full, mat_sb = st["ksum_full"], st["mat_sb"]
            nc.vector.reduce_sum(
                out=r(ksum_full), in_=st["ks_parts"], axis=mybir.AxisListType.X
            )
            if use_bv:
                ksr_ps = pp_small.tile([1, 66], f32, tag="small",
                                       name=f"ksr{b}")
                nc.tensor.matmul(
                    ksr_ps, r(ksum_full), r(ident[0:65, 0:66]),
                    start=True, stop=True,
                )
                nc.vector.tensor_copy(
                    out=r(st["ksumn_row"]), in_=ksr_ps[0:1, 0:65]
                )

            nc.vector.tensor_copy(out=r(st["p_sb"]), in_=st["p_ps"])
            pt_ps = pp_small.tile([128, 264], f32, tag="small", name=f"pt{b}")
            for k in range(KC):
                nc.tensor.transpose(
                    r(pt_ps[:, 66 * k:66 * (k + 1)]),
                    r(st["p_sb"][:, 128 * k:128 * (k + 1)]),
                    r(ident[0:65, 0:66]),
                )
            nc.vector.tensor_copy(
                out=r(st["pt_sb"]),
                in_=pt_ps[:].rearrange("p (k c) -> p k c", c=66)[:, :, 0:65],
            )
            mat_ps = pp_acc.tile([65, 512], f32, tag="acc", name=f"mat_ps{b}")
            for k in range(KC):
                nc.tensor.matmul(
                    mat_ps, r(st["pt_sb"][:, k, :]), r(wvt[:, k, :]),
                    start=(k == 0), stop=(k == KC - 1 and not use_bv),
                    skip_group_check=True,
                )
            if use_bv:
                nc.tensor.matmul(
                    mat_ps, r(st["ksumn_row"]), r(bv), start=False, stop=True,
                    skip_group_check=True,
                )
            # gamma folded into matrix_ext during the psum->sbuf move
            nc.vector.tensor_scalar_mul(
                out=r(mat_sb), in0=mat_ps, scalar1=gam128[0:65, :]
            )

            # ks8: column nb holds ksum in slot nb of each 8-block
            for nb in range(NB):
                nc.vector.tensor_copy(
                    out=r(ks8[:, 8 * nb + nb:8 * nb + nb + 1]),
                    in_=ksum_full[0:64, :],
                )
            # s8[i, :] = ksum^T Q_raw(chunk i), stacked via one-hot lhsT
            s8_ps = pp_acc.tile([8, 512], f32, tag="acc", name=f"s8{b}")
            for nb in range(NB):
                sl = slice(512 * nb, 512 * (nb + 1))
                nc.tensor.matmul(
                    s8_ps, r(ks8[:, 8 * nb:8 * (nb + 1)]), r(q_raw[0:64, sl]),
                    start=(nb == 0), stop=(nb == NB - 1), skip_group_check=True,
                )

            # u = 1/(N*nq + s); tailor = nq*u
            # ut: rows 0-7 = u per chunk, rows 32-39 = tailor per chunk
            nq8 = work.tile([8, 512], f32, tag="nq8", name=f"nq8_{b}")
            nc.scalar.sqrt(out=nq8, in_=st["n2q8_ps"])
            t1 = work.tile([8, 512], f32, tag="t1", name=f"t1_{b}")
            nc.vector.scalar_tensor_tensor(
                out=t1, in0=nq8, scalar=float(N), in1=s8_ps,
                op0=alu.mult, op1=alu.add,
            )
            ut = st["ut"]
            nc.vector.reciprocal(out=r(ut[0:8, :]), in_=t1)
            nc.vector.tensor_mul(out=r(ut[32:40, :]), in0=nq8, in1=ut[0:8, :])

        def emit_B_t2(b, st, nb):
            sl = slice(512 * nb, 512 * (nb + 1))
            # T2 rows 0-63 = u(chunk nb), row 64 = tailor(chunk nb)
            t2_ps = pp_big.tile([65, 512], f32, tag="big", name=f"t2_{b}_{nb}")
            nc.tensor.matmul(
                t2_ps, r(selab[:, 65 * nb:65 * (nb + 1)]), r(st["ut"]),
                start=True, stop=True,
            )
            qns = work.tile([65, 512], f32, tag="qns", bufs=4,
                            name=f"qns{b}_{nb}")
            nc.vector.tensor_mul(out=r(qns), in0=st["q_raw"][:, sl], in1=t2_ps)
            st["qns"][nb] = qns

        def emit_B_out(b, st, nb):
            sl = slice(512 * nb, 512 * (nb + 1))
            qns = st["qns"].pop(nb)
            for cb in range(KC):
                o_ps = pp_big.tile([128, 512], f32, tag="big",
                                   name=f"o_ps{b}_{nb}_{cb}")
                nc.tensor.matmul(
                    o_ps, r(st["mat_sb"][:, 128 * cb:128 * (cb + 1)]), r(qns),
                    start=True, stop=True,
                )
                o_sb = outp.tile([128, 512], f32, tag="o",
                                 name=f"o_sb{b}_{nb}_{cb}")
                nc.scalar.copy(out=o_sb, in_=o_ps)
                nc.sync.dma_start(
                    out=out_d[b, 128 * cb:128 * (cb + 1), sl], in_=o_sb
                )

        # Software pipeline: batch b stage A interleaves with batch b-1
        # stage B; P/ks and out matmuls trail their producers by one chunk so
        # the in-order PE stream never waits on DVE/ACT chains.
        states = {}
        for b in range(BLOC):
            states[b] = alloc_state(b)
            for nb in range(NB):
                emit_A_chunk(b, states[b], nb)
                if nb > 0:
                    emit_P_chunk(b, states[b], nb - 1)
                if b > 0:
                    emit_B_t2(b - 1, states[b - 1], nb)
                    if nb > 0:
                        emit_B_out(b - 1, states[b - 1], nb - 1)
            emit_P_chunk(b, states[b], NB - 1)
            if b > 0:
                emit_B_out(b - 1, states[b - 1], NB - 1)
            emit_A2(b, states[b])
        stl = states[BLOC - 1]
        for nb in range(NB):
            emit_B_t2(BLOC - 1, stl, nb)
            if nb > 0:
                emit_B_out(BLOC - 1, stl, nb - 1)
        emit_B_out(BLOC - 1, stl, NB - 1)

    _split_multi_waits(nc)
    return nc


_CACHE = {}


def _get_module(use_bqk, use_bv):
    key = (use_bqk, use_bv)
    if key not in _CACHE:
        _CACHE[key] = _build_module(*key)
    return _CACHE[key]


def _host_inputs(x, Wq, bq, Wk, bk, Wv, bv, gamma):
    x = np.ascontiguousarray(np.asarray(x, dtype=np.float32)).reshape(B, C, N)
    Wq = np.asarray(Wq, dtype=np.float32)
    Wk = np.asarray(Wk, dtype=np.float32)
    Wv = np.asarray(Wv, dtype=np.float32)
    bq = np.asarray(bq, dtype=np.float32)
    bk = np.asarray(bk, dtype=np.float32)
    bvv = np.asarray(bv, dtype=np.float32)
    gamma = np.asarray(gamma, dtype=np.float32)

    wqk = np.concatenate([Wq, Wk], axis=0)            # [128, 512]
    wqkt = np.ascontiguousarray(
        wqk.T.reshape(KC, 128, 128).transpose(1, 0, 2)
    )                                                 # [128, KC, 128]
    wvt = np.ascontiguousarray(
        Wv.T.reshape(KC, 128, 512).transpose(1, 0, 2)
    )                                                 # [128, KC, 512]
    bqkr = np.concatenate([bq, bk]).reshape(1, 128)
    bvr = bvv.reshape(1, 512)
    gam = gamma.reshape(1, 1).astype(np.float32)
    sel8 = np.zeros((64, 64), np.float32)
    for nb in range(8):
        sel8[:, 8 * nb + nb] = 1.0
    selab = np.zeros((40, 520), np.float32)
    for nb in range(8):
        selab[nb, 65 * nb:65 * nb + 64] = 1.0
        selab[32 + nb, 65 * nb + 64] = 1.0
    onesn = np.ones((1, 512), np.float32)
    onesc2 = np.ones((128, 2), np.float32)
    ident = np.eye(128, dtype=np.float32)

    shared = dict(
        wqkt=wqkt, wvt=wvt, bqk=bqkr, bv=bvr, gamma=gam,
        sel8=sel8, selab=selab, onesn=onesn, onesc2=onesc2,
        ident=ident,
    )
    in_maps = []
    for c in range(NCORES):
        m = dict(shared)
        m["x"] = np.ascontiguousarray(x[c * BLOC:(c + 1) * BLOC])
        in_maps.append(m)
    return in_maps


def run_on_device(in_maps, **kw):
    from concourse.bass_utils import run_bass_kernel_spmd

    m = in_maps[0]
    use_bqk = bool(np.any(m["bqk"]))
    use_bv = bool(np.any(m["bv"]))
    nc = _get_module(use_bqk, use_bv)
    return run_bass_kernel_spmd(nc, in_maps, core_ids=list(range(NCORES)), **kw)


def kernel(x, Wq, bq, Wk, bk, Wv, bv, gamma):
    in_maps = _host_inputs(x, Wq, bq, Wk, bk, Wv, bv, gamma)
    res = run_on_device(in_maps)
    out = np.concatenate([r["out"] for r in res.results], axis=0)
    return out.reshape(B, C, H, W).astype(np.float32)


# revision 28
# speedup vs baseline: 1.0058x; 1.0003x over previous
"""Trainium2 Bass kernel for nn_Attention_33638183862624 (linear/Taylor-softmax
attention). Data-parallel over batch: 16 batches -> 8 NeuronCores, 2 each.

Math per batch (C=512, N=4096, CQK=64), x flattened to [C, N]:
  Q = Wq x + bq; K = Wk x + bk           (Q,K: [64, N])
  Qn = Q / ||Q||_col; Kn = K / ||K||_col
  ksum = sum_n Kn[:, n]                  [64]
  denom = N + Qn^T ksum; tailor = 1/denom
  V = Wv x + bv                          (never materialized; fused:)
  matrix = Kn V^T = (Kn x^T) Wv^T + ksum bv^T            [64, 512]
  vsum   = V 1_N  = Wv (x 1_N) + N bv                    [512]
  out[c,n] = gamma * tailor[n] * (vsum[c] + sum_m matrix[m,c] Qn[m,n])

Key device-side identities:
  * Q is kept RAW; with s = ksum^T Q_raw and nq = ||Q||_col:
      u := rq*tailor = 1/(N*nq + s), and tailor = nq*u.
    The final rhs is [Q_raw; nq] * broadcast(u) so one elementwise reciprocal
    on a [8, 512] tile per batch covers all of Q-normalize + tailor.
  * K is normalized in TRANSPOSED layout (per-partition 1/||K|| scalars,
    reciprocal on [128, 1] -> all 128 DVE lanes busy).
  * matrix/vsum/ksum come from one accumulated (Kn_ext @ x^T) product plus a
    small @ Wv^T stage; V itself is never computed.

All matmuls run as float32r (1-pass reduced-precision fp32, full PE rate at
free-dim 512). n-contractions use PE transposes of x and K chunks.
"""

import numpy as np

B, C, H, W = 16, 512, 64, 64
N = H * W          # 4096
CQK = C // 8       # 64
NCORES = 8
BLOC = B // NCORES  # 2 batches per core
NB = N // 512       # 8 n-chunks of 512
KC = C // 128       # 4 channel chunks of 128


# ---------------------------------------------------------------------------
# Walrus workaround: this container's walrus rejects >1 sync wait per
# instruction ("Too many sync wait commands"). (1) patch the TileContext tail
# drain to carry its waits on single-wait NOPs; (2) post-pass that rewrites
# any instruction with k>1 waits into k-1 single-wait NOPs + the instruction.
# ---------------------------------------------------------------------------

def _apply_tile_patches():
    import concourse.tile as tile
    from concourse import mybir
    from concourse.vector_clock import ScopedClock

    if getattr(tile.TileContext, "_drain_patched", False):
        return

    def _patched_drain_and_barrier(self, tick_clock, wait_clock):
        nop = self.nc.sync.nop(nofuse=True, hint="tail_drain_waits")
        wait_clock.add_sem_waits(
            nop.ins, ScopedClock({None: tick_clock.global_clock})
        )
        si = nop.ins.sync_info
        if si is not None and len(si.on_wait) > 1:
            waits = list(si.on_wait)
            nop.ins.sync_info = mybir.SyncInfo(on_wait=waits[:1], on_update=[])
            rest = waits[1:]
            while rest:
                n2 = self.nc.sync.nop(nofuse=True, hint="tail_drain_waits")
                n2.ins.sync_info = mybir.SyncInfo(on_wait=rest[:1], on_update=[])
                rest = rest[1:]
        self.nc.sync.drain()
        self.nc.all_engine_barrier()
        assert self.sems is not None
        popped = self.nc._tile_sem_poison_stack.pop()
        assert popped is self._sem_poison
        self.nc.clear_and_free_semaphores(list(self.sems.allocated().values()))
        self.nc.all_engine_barrier()

    tile.TileContext._drain_and_barrier = _patched_drain_and_barrier
    tile.TileContext._drain_patched = True


def _split_multi_waits(nc):
    from concourse import mybir

    counter = [0]
    for f in nc.m.functions:
        for bb in f.blocks:
            insts = bb.instructions
            if not any(
                i.sync_info is not None and len(i.sync_info.on_wait) > 1
                for i in insts
            ):
                continue
            new = []
            for ins in insts:
                si = ins.sync_info
                if si is not None and len(si.on_wait) > 1:
                    waits = list(si.on_wait)
                    for w in waits[:-1]:
                        counter[0] += 1
                        nop = mybir.InstNoOp(
                            name=f"I-wsplit-{counter[0]}", ins=[], outs=[]
                        )
                        nop.engine = ins.engine
                        nop.sync_info = mybir.SyncInfo(on_wait=[w], on_update=[])
                        new.append(nop)
                    ins.sync_info = mybir.SyncInfo(
                        on_wait=[waits[-1]], on_update=list(si.on_update)
                    )
                new.append(ins)
            bb.instructions = new


# ---------------------------------------------------------------------------
# Kernel body
# ---------------------------------------------------------------------------

def _build_module(use_bqk=True, use_bv=True):
    import concourse.bass as bass
    import concourse.tile as tile
    from concourse import mybir

    _apply_tile_patches()
    f32 = mybir.dt.float32
    f32r = mybir.dt.float32r
    alu = mybir.AluOpType
    r = lambda ap: ap.bitcast(f32r)

    nc = bass.Bass("TRN2", target_bir_lowering=False, debug=False)

    x_d = nc.dram_tensor("x", [BLOC, C, N], f32, kind="ExternalInput").ap()
    wqkt_d = nc.dram_tensor("wqkt", [128, KC, 128], f32, kind="ExternalInput").ap()
    wvt_d = nc.dram_tensor("wvt", [128, KC, 512], f32, kind="ExternalInput").ap()
    bqk_d = nc.dram_tensor("bqk", [1, 128], f32, kind="ExternalInput").ap()
    bv_d = nc.dram_tensor("bv", [1, 512], f32, kind="ExternalInput").ap()
    gam_d = nc.dram_tensor("gamma", [1, 1], f32, kind="ExternalInput").ap()
    sel8_d = nc.dram_tensor("sel8", [64, 64], f32, kind="ExternalInput").ap()
    selab_d = nc.dram_tensor("selab", [40, 520], f32, kind="ExternalInput").ap()
    onesn_d = nc.dram_tensor("onesn", [1, 512], f32, kind="ExternalInput").ap()
    onesc2_d = nc.dram_tensor("onesc2", [128, 2], f32, kind="ExternalInput").ap()
    ident_d = nc.dram_tensor("ident", [128, 128], f32, kind="ExternalInput").ap()
    out_d = nc.dram_tensor("out", [BLOC, C, N], f32, kind="ExternalOutput").ap()

    from contextlib import ExitStack

    with tile.TileContext(nc) as tc, ExitStack() as ctx, \
            nc.allow_low_precision(reason="float32r views are bit-compatible fp32"):
        consts = ctx.enter_context(tc.tile_pool(name="consts", bufs=1))
        xpool = ctx.enter_context(tc.tile_pool(name="xpool", bufs=2))
        batchp = ctx.enter_context(tc.tile_pool(name="batchp", bufs=2))
        work = ctx.enter_context(tc.tile_pool(name="work", bufs=3))
        outp = ctx.enter_context(tc.tile_pool(name="outp", bufs=6))
        pp_big = ctx.enter_context(tc.tile_pool(name="pp_big", bufs=5, space="PSUM"))
        pp_acc = ctx.enter_context(tc.tile_pool(name="pp_acc", bufs=2, space="PSUM"))
        pp_small = ctx.enter_context(
            tc.tile_pool(name="pp_small", bufs=1, space="PSUM")
        )

        # ---- constants ----
        wqkt = consts.tile([128, KC, 128], f32)
        nc.sync.dma_start(out=r(wqkt), in_=r(wqkt_d))
        wvt = consts.tile([128, KC, 512], f32)
        nc.sync.dma_start(out=r(wvt), in_=r(wvt_d))
        bqk = consts.tile([1, 128], f32)
        nc.sync.dma_start(out=r(bqk), in_=r(bqk_d))
        bv = consts.tile([1, 512], f32)
        nc.sync.dma_start(out=r(bv), in_=r(bv_d))
        sel8 = consts.tile([64, 64], f32)
        nc.sync.dma_start(out=r(sel8), in_=r(sel8_d))
        selab = consts.tile([40, 520], f32)
        nc.sync.dma_start(out=r(selab), in_=r(selab_d))
        onesn = consts.tile([1, 512], f32)
        nc.sync.dma_start(out=r(onesn), in_=r(onesn_d))
        onesc2 = consts.tile([128, 2], f32)
        nc.sync.dma_start(out=r(onesc2), in_=r(onesc2_d))
        ident = consts.tile([128, 128], f32)
        nc.sync.dma_start(out=r(ident), in_=r(ident_d))
        gam128 = consts.tile([128, 1], f32)
        nc.sync.dma_start(
            out=gam128,
            in_=bass.AP(
                tensor=gam_d.tensor, offset=gam_d.offset,
                ap=[[0, 128], [1, 1]],
            ),
        )

        def alloc_state(b):
            st = {}
            st["q_raw"] = batchp.tile([65, N], f32, tag="q_raw",
                                      name=f"q_raw{b}")
            nc.gpsimd.memset(st["q_raw"][64:65, :], 1.0)
            st["ks8"] = batchp.tile([64, 64], f32, tag="ks8", name=f"ks8_{b}")
            nc.gpsimd.memset(st["ks8"], 0.0)
            st["ks_parts"] = batchp.tile([65, NB], f32, tag="ks_parts",
                                         name=f"ks_parts{b}")
            st["ksum_full"] = batchp.tile([65, 1], f32, tag="ksum_full",
                                          name=f"ksum_full{b}")
            st["ksumn_row"] = batchp.tile([1, 65], f32, tag="ksumn_row",
                                          name=f"ksumn_row{b}")
            st["p_sb"] = batchp.tile([65, 512], f32, tag="p_sb",
                                     name=f"p_sb{b}")
            st["pt_sb"] = batchp.tile([128, KC, 65], f32, tag="pt_sb",
                                      name=f"pt_sb{b}")
            st["mat_sb"] = batchp.tile([65, 512], f32, tag="mat_sb",
                                       name=f"mat_sb{b}")
            st["ut"] = batchp.tile([40, 512], f32, tag="ut", name=f"ut{b}")
            st["p_ps"] = pp_acc.tile([65, 512], f32, tag="acc",
                                     name=f"p_ps{b}")
            st["n2q8_ps"] = pp_acc.tile([8, 512], f32, tag="acc",
                                        name=f"n2q8_ps{b}")
            st["xh"] = {}
            st["xt"] = {}
            st["knt"] = {}
            st["qns"] = {}
            return st

        def emit_A_chunk(b, st, nb):
            half, col = nb // 4, (nb % 4) * 512
            if nb % 4 == 0:
                for k in range(KC):
                    st["xh"][k] = xpool.tile([128, 2048], f32, tag=f"x{k}",
                                             name=f"xh{k}_{b}_{half}")
                    for piece in range(2):
                        nc.sync.dma_start(
                            out=r(st["xh"][k][:, 1024 * piece:1024 * (piece + 1)]),
                            in_=r(x_d[b, 128 * k:128 * (k + 1),
                                      2048 * half + 1024 * piece:
                                      2048 * half + 1024 * (piece + 1)]),
                        )
            xh = st["xh"]

            # QK = Wqk x + bqk -> psum [128, 512] (rows 0-63 Q, 64-127 K)
            qk_ps = pp_big.tile([128, 512], f32, tag="big", name=f"qk{b}_{nb}")
            for k in range(KC):
                nc.tensor.matmul(
                    qk_ps, r(wqkt[:, k, :]), r(xh[k][:, col:col + 512]),
                    start=(k == 0), stop=(k == KC - 1 and not use_bqk),
                )
            if use_bqk:
                nc.tensor.matmul(qk_ps, r(bqk), r(onesn), start=False, stop=True)

            # stash raw Q; K to sbuf for transposing
            nc.vector.tensor_copy(
                out=r(st["q_raw"][0:64, 512 * nb:512 * (nb + 1)]),
                in_=qk_ps[0:64, :],
            )
            k_sb = work.tile([64, 512], f32, tag="k_sb", bufs=4, name=f"k_sb{b}_{nb}")
            nc.vector.tensor_copy(out=r(k_sb), in_=qk_ps[64:128, :])
            sq_sb = work.tile([64, 512], f32, tag="sq", bufs=4, name=f"sq{b}_{nb}")
            nc.scalar.square(out=r(sq_sb), in_=qk_ps[0:64, :])

            # x^T chunks (depend only on xh -> keep PE stream dense here
            # while DVE/ACT produce k_sb/sq)
            st["xt"][nb] = []
            for j in range(4):
                xt_ps = pp_big.tile([128, 512], f32, tag="big",
                                    name=f"xt{b}_{nb}_{j}")
                for k in range(KC):
                    nc.tensor.transpose(
                        r(xt_ps[:, 128 * k:128 * (k + 1)]),
                        r(xh[k][:, col + 128 * j:col + 128 * (j + 1)]),
                        r(ident),
                    )
                xt_sb = work.tile([128, 512], f32, tag="xt", bufs=9,
                                  name=f"xtsb{b}_{nb}_{j}")
                if j % 2 == 0:
                    nc.vector.tensor_copy(out=r(xt_sb), in_=xt_ps)
                else:
                    nc.scalar.copy(out=r(xt_sb), in_=xt_ps)
                st["xt"][nb].append(xt_sb)

            # K^T chunks (raw), then normalize per-partition
            kt_ps = pp_big.tile([128, 256], f32, tag="big", name=f"kt{b}_{nb}")
            for j in range(4):
                nc.tensor.transpose(
                    r(kt_ps[:, 64 * j:64 * (j + 1)]),
                    r(k_sb[:, 128 * j:128 * (j + 1)]),
                    r(ident[0:64, 0:64]),
                )
            # Q column norms^2 -> accumulate into row nb of n2q8_ps
            nc.tensor.matmul(
                st["n2q8_ps"], r(sel8[:, 8 * nb:8 * (nb + 1)]), r(sq_sb),
                start=(nb == 0), stop=(nb == NB - 1), skip_group_check=True,
            )
            knt_raw = work.tile([128, 256], f32, tag="knt_raw",
                                name=f"knt_raw{b}_{nb}")
            nc.scalar.copy(out=knt_raw, in_=kt_ps)
            knt_sb = work.tile([128, 4, 65], f32, tag="knt", bufs=4,
                               name=f"knt{b}_{nb}")
            for j in range(4):
                ksq = work.tile([128, 64], f32, tag="ksq", name=f"ksq{b}_{nb}_{j}")
                nc.scalar.square(out=ksq, in_=knt_raw[:, 64 * j:64 * (j + 1)])
                nk2 = work.tile([128, 1], f32, tag="nk2", name=f"nk2{b}_{nb}_{j}")
                nc.vector.reduce_sum(out=nk2, in_=ksq, axis=mybir.AxisListType.X)
                nkt = work.tile([128, 1], f32, tag="nkt", name=f"nkt{b}_{nb}_{j}")
                nc.scalar.sqrt(out=nkt, in_=nk2)
                rkt = work.tile([128, 1], f32, tag="rkt", name=f"rkt{b}_{nb}_{j}")
                nc.vector.reciprocal(out=rkt, in_=nkt)
                nc.vector.tensor_scalar_mul(
                    out=r(knt_sb[:, j, 0:64]),
                    in0=knt_raw[:, 64 * j:64 * (j + 1)], scalar1=rkt,
                )
            nc.gpsimd.memset(knt_sb[:, :, 64:65], 1.0)
            st["knt"][nb] = knt_sb

        def emit_P_chunk(b, st, nb):
            # deferred one chunk so the knt/xt producer chains have slack
            knt_sb = st["knt"].pop(nb)
            xts = st["xt"].pop(nb)
            ks_ps = pp_small.tile([65, 2], f32, tag="small", name=f"ksp{b}_{nb}")
            for j in range(4):
                nc.tensor.matmul(
                    st["p_ps"], r(knt_sb[:, j, :]), r(xts[j]),
                    start=(nb == 0 and j == 0),
                    stop=(nb == NB - 1 and j == 3),
                    skip_group_check=True,
                )
                nc.tensor.matmul(
                    ks_ps, r(knt_sb[:, j, :]), r(onesc2),
                    start=(j == 0), stop=(j == 3),
                    skip_group_check=True,
                )
            nc.vector.tensor_copy(
                out=st["ks_parts"][:, nb:nb + 1], in_=ks_ps[:, 0:1]
            )

        def emit_A2(b, st):
            q_raw, ks8 = st["q_raw"], st["ks8"]
            ksum_full, mat_sb = st["ksum_full"], st["mat_sb"]
            nc.vector.reduce_sum(
                out=r(ksum_full), in_=st["ks_parts"], axis=mybir.AxisListType.X
            )
            if use_bv:
                ksr_ps = pp_small.tile([1, 66], f32, tag="small",
                                       name=f"ksr{b}")
                nc.tensor.matmul(
                    ksr_ps, r(ksum_full), r(ident[0:65, 0:66]),
                    start=True, stop=True,
                )
                nc.vector.tensor_copy(
                    out=r(st["ksumn_row"]), in_=ksr_ps[0:1, 0:65]
                )

            nc.vector.tensor_copy(out=r(st["p_sb"]), in_=st["p_ps"])
            pt_ps = pp_small.tile([128, 264], f32, tag="small", name=f"pt{b}")
            for k in range(KC):
                nc.tensor.transpose(
                    r(pt_ps[:, 66 * k:66 * (k + 1)]),
                    r(st["p_sb"][:, 128 * k:128 * (k + 1)]),
                    r(ident[0:65, 0:66]),
                )
            nc.vector.tensor_copy(
                out=r(st["pt_sb"]),
                in_=pt_ps[:].rearrange("p (k c) -> p k c", c=66)[:, :, 0:65],
            )
            mat_ps = pp_acc.tile([65, 512], f32, tag="acc", name=f"mat_ps{b}")
            for k in range(KC):
                nc.tensor.matmul(
                    mat_ps, r(st["pt_sb"][:, k, :]), r(wvt[:, k, :]),
                    start=(k == 0), stop=(k == KC - 1 and not use_bv),
                    skip_group_check=True,
                )
            if use_bv:
                nc.tensor.matmul(
                    mat_ps, r(st["ksumn_row"]), r(bv), start=False, stop=True,
                    skip_group_check=True,
                )
            # gamma folded into matrix_ext during the psum->sbuf move
            nc.vector.tensor_scalar_mul(
                out=r(mat_sb), in0=mat_ps, scalar1=gam128[0:65, :]
            )

            # ks8: column nb holds ksum in slot nb of each 8-block
            for nb in range(NB):
                nc.vector.tensor_copy(
                    out=r(ks8[:, 8 * nb + nb:8 * nb + nb + 1]),
                    in_=ksum_full[0:64, :],
                )
            # s8[i, :] = ksum^T Q_raw(chunk i), stacked via one-hot lhsT
            s8_ps = pp_acc.tile([8, 512], f32, tag="acc", name=f"s8{b}")
            for nb in range(NB):
                sl = slice(512 * nb, 512 * (nb + 1))
                nc.tensor.matmul(
                    s8_ps, r(ks8[:, 8 * nb:8 * (nb + 1)]), r(q_raw[0:64, sl]),
                    start=(nb == 0), stop=(nb == NB - 1), skip_group_check=True,
                )

            # u = 1/(N*nq + s); tailor = nq*u
            # ut: rows 0-7 = u per chunk, rows 32-39 = tailor per chunk
            nq8 = work.tile([8, 512], f32, tag="nq8", name=f"nq8_{b}")
            nc.scalar.sqrt(out=nq8, in_=st["n2q8_ps"])
            t1 = work.tile([8, 512], f32, tag="t1", name=f"t1_{b}")
            nc.vector.scalar_tensor_tensor(
                out=t1, in0=nq8, scalar=float(N), in1=s8_ps,
                op0=alu.mult, op1=alu.add,
            )
            ut = st["ut"]
            nc.vector.reciprocal(out=r(ut[0:8, :]), in_=t1)
            nc.vector.tensor_mul(out=r(ut[32:40, :]), in0=nq8, in1=ut[0:8, :])

        def emit_B_t2(b, st, nb):
            sl = slice(512 * nb, 512 * (nb + 1))
            # T2 rows 0-63 = u(chunk nb), row 64 = tailor(chunk nb)
            t2_ps = pp_big.tile([65, 512], f32, tag="big", name=f"t2_{b}_{nb}")
            nc.tensor.matmul(
                t2_ps, r(selab[:, 65 * nb:65 * (nb + 1)]), r(st["ut"]),
                start=True, stop=True,
            )
            qns = work.tile([65, 512], f32, tag="qns", bufs=4,
                            name=f"qns{b}_{nb}")
            nc.vector.tensor_mul(out=r(qns), in0=st["q_raw"][:, sl], in1=t2_ps)
            st["qns"][nb] = qns

        def emit_B_out(b, st, nb):
            sl = slice(512 * nb, 512 * (nb + 1))
            qns = st["qns"].pop(nb)
            for cb in range(KC):
                o_ps = pp_big.tile([128, 512], f32, tag="big",
                                   name=f"o_ps{b}_{nb}_{cb}")
                nc.tensor.matmul(
                    o_ps, r(st["mat_sb"][:, 128 * cb:128 * (cb + 1)]), r(qns),
                    start=True, stop=True,
                )
                o_sb = outp.tile([128, 512], f32, tag="o",
                                 name=f"o_sb{b}_{nb}_{cb}")
                nc.scalar.copy(out=o_sb, in_=o_ps)
                nc.sync.dma_start(
                    out=out_d[b, 128 * cb:128 * (cb + 1), sl], in_=o_sb
                )

        # Software pipeline: batch b stage A interleaves with batch b-1
        # stage B; P/ks and out matmuls trail their producers by one chunk so
        # the in-order PE stream never waits on DVE/ACT chains.
        states = {}
        for b in range(BLOC):
            states[b] = alloc_state(b)
            for nb in range(NB):
                emit_A_chunk(b, states[b], nb)
                if nb > 0:
                    emit_P_chunk(b, states[b], nb - 1)
                if b > 0:
                    emit_B_t2(b - 1, states[b - 1], nb)
                    if nb > 0:
                        emit_B_out(b - 1, states[b - 1], nb - 1)
            emit_P_chunk(b, states[b], NB - 1)
            if b > 0:
                emit_B_out(b - 1, states[b - 1], NB - 1)
            emit_A2(b, states[b])
        stl = states[BLOC - 1]
        for nb in range(NB):
            emit_B_t2(BLOC - 1, stl, nb)
            if nb > 0:
                emit_B_out(BLOC - 1, stl, nb - 1)
        emit_B_out(BLOC - 1, stl, NB - 1)

    _split_multi_waits(nc)
    return nc


_CACHE = {}


def _get_module(use_bqk, use_bv):
    key = (use_bqk, use_bv)
    if key not in _CACHE:
        _CACHE[key] = _build_module(*key)
    return _CACHE[key]


def _host_inputs(x, Wq, bq, Wk, bk, Wv, bv, gamma):
    x = np.ascontiguousarray(np.asarray(x, dtype=np.float32)).reshape(B, C, N)
    Wq = np.asarray(Wq, dtype=np.float32)
    Wk = np.asarray(Wk, dtype=np.float32)
    Wv = np.asarray(Wv, dtype=np.float32)
    bq = np.asarray(bq, dtype=np.float32)
    bk = np.asarray(bk, dtype=np.float32)
    bvv = np.asarray(bv, dtype=np.float32)
    gamma = np.asarray(gamma, dtype=np.float32)

    wqk = np.concatenate([Wq, Wk], axis=0)            # [128, 512]
    wqkt = np.ascontiguousarray(
        wqk.T.reshape(KC, 128, 128).transpose(1, 0, 2)
    )                                                 # [128, KC, 128]
    wvt = np.ascontiguousarray(
        Wv.T.reshape(KC, 128, 512).transpose(1, 0, 2)
    )                                                 # [128, KC, 512]
    bqkr = np.concatenate([bq, bk]).reshape(1, 128)
    bvr = bvv.reshape(1, 512)
    gam = gamma.reshape(1, 1).astype(np.float32)
    sel8 = np.zeros((64, 64), np.float32)
    for nb in range(8):
        sel8[:, 8 * nb + nb] = 1.0
    selab = np.zeros((40, 520), np.float32)
    for nb in range(8):
        selab[nb, 65 * nb:65 * nb + 64] = 1.0
        selab[32 + nb, 65 * nb + 64] = 1.0
    onesn = np.ones((1, 512), np.float32)
    onesc2 = np.ones((128, 2), np.float32)
    ident = np.eye(128, dtype=np.float32)

    shared = dict(
        wqkt=wqkt, wvt=wvt, bqk=bqkr, bv=bvr, gamma=gam,
        sel8=sel8, selab=selab, onesn=onesn, onesc2=onesc2,
        ident=ident,
    )
    in_maps = []
    for c in range(NCORES):
        m = dict(shared)
        m["x"] = np.ascontiguousarray(x[c * BLOC:(c + 1) * BLOC])
        in_maps.append(m)
    return in_maps


def run_on_device(in_maps, **kw):
    from concourse.bass_utils import run_bass_kernel_spmd

    m = in_maps[0]
    use_bqk = bool(np.any(m["bqk"]))
    use_bv = bool(np.any(m["bv"]))
    nc = _get_module(use_bqk, use_bv)
    return run_bass_kernel_spmd(nc, in_maps, core_ids=list(range(NCORES)), **kw)


def kernel(x, Wq, bq, Wk, bk, Wv, bv, gamma):
    in_maps = _host_inputs(x, Wq, bq, Wk, bk, Wv, bv, gamma)
    res = run_on_device(in_maps)
    out = np.concatenate([r["out"] for r in res.results], axis=0)
    return out.reshape(B, C, H, W).astype(np.float32)
